# revision 28
# baseline (speedup 1.0000x reference)
"""DCNv3-1D fused Trainium2 kernel (8-core batch-parallel SPMD), v3.

Reference semantics (per batch row, N rows sharded 1/core):
  x_proj = x @ W_in + b_in
  y      = depthwise_conv3(x) + conv_b ; LN over C ; GELU -> x_feat
  offset = x_feat @ W_off + b_off ; mask = softmax_K(x_feat @ W_mask + b_mask)
  loc    = l + dil_grid + offset (mod L); bilinear sample x_proj along L
  out    = (sum_k mask * sampled) @ W_out + b_out

v3 dataflow: everything that can leave the compute engines does.
 - x_proj is produced directly transposed (xpT[c, l]) so the 5-tap band apply
   is bf16 2x-mode vector ops over shifted column slices (wrap halo cols).
 - offset/mask logits are computed transposed (W_om stationary, 16 matmuls),
   evacuated once to bf16 and DMA-transposed back to l-layout for the
   full-width band-weight math.
 - band weights a[l, g, s] -> abf[l-part, t, q=g*8+s] (bf16, zero-padded) ->
   xbar transpose -> aT[q, l] -> DRAM bounce -> ONE replicated DMA per
   (hp, chunk) expands groups to channels (ws_blk[c, s, l]); zero matmuls,
   zero PSUM evacuation for the expansion.
 - l==0/1/L-2/L-1 zero-pad edge masking is applied as 4 single-partition
   fixups instead of full-width compares.
 - output projection consumes sampT (lhsT) directly; result is written
   channel-major bf16; the host transposes/upcasts.
"""

import numpy as np

import concourse.bacc as bacc
import concourse.bass as bass
import concourse.mybir as mybir
from concourse.tile import TileContext
from concourse.bass_utils import run_bass_kernel_spmd

N, L, C, G, K = 8, 4096, 256, 8, 3
GC = C // G
T = L // 128          # 32 l-tiles
H = C // 128          # 2 channel halves
SMIN, SMAX = -2, 2    # shift band (covers |offset| < 2 - dilation tap reach)
NS = SMAX - SMIN + 1  # 5
NSP = 8               # padded s-stride inside abf (q = g*NSP + s)
LN_EPS = 1e-6

F32 = mybir.dt.float32
BF16 = mybir.dt.bfloat16
I32 = mybir.dt.int32
Alu = mybir.AluOpType
Act = mybir.ActivationFunctionType

_CACHE = {}


def _build(flags):
    nc = bacc.Bacc("TRN2", target_bir_lowering=False, debug=False, num_devices=8)

    # ---- DRAM I/O ----
    xbf = nc.dram_tensor("xbf", [L, C], BF16, kind="ExternalInput")
    NE = 2 * NS * 128
    NBF = H * C + H * 2 * G * K + H * C + K * H * 128 + 128 + 128 + NE
    cbf = nc.dram_tensor("cbf", [128, NBF], BF16, kind="ExternalInput")
    NF3 = 2 * G * K + T + 2
    cf3 = nc.dram_tensor("cf3", [128, NF3], F32, kind="ExternalInput")
    out_d = nc.dram_tensor("out", [C, L], BF16, kind="ExternalOutput")
    if flags["has_bin"]:
        binc = nc.dram_tensor("binc", [128, H], F32, kind="ExternalInput")
    if flags["has_convb"]:
        convb = nc.dram_tensor("convb", [128, H, 1], F32, kind="ExternalInput")
    if flags["has_ln"]:
        lngb = nc.dram_tensor("lngb", [128, H, 2], BF16, kind="ExternalInput")
    if flags["has_bout"]:
        boutc = nc.dram_tensor("boutc", [128, H], F32, kind="ExternalInput")

    with TileContext(nc) as tc, nc.allow_low_precision(reason="bf16 kernel by design"):
        _emit(nc, tc, flags, locals())
    nc.compile()
    return nc


def _emit(nc, tc, flags, dram):
    from contextlib import ExitStack

    ctx = ExitStack()
    with ctx:
        from contextlib import ExitStack as _ES
        consts = ctx.enter_context(tc.tile_pool(name="consts", bufs=1))
        xTp = ctx.enter_context(tc.tile_pool(name="xTp", bufs=2))
        arena = ctx.enter_context(tc.tile_pool(name="arena", bufs=4))
        pers = ctx.enter_context(tc.tile_pool(name="pers", bufs=1))
        ysqp = ctx.enter_context(tc.tile_pool(name="ysqp", bufs=6))
        psp = ctx.enter_context(tc.tile_pool(name="psp", bufs=4, space="PSUM"))
        early = _ES()
        b16 = early.enter_context(tc.tile_pool(name="b16", bufs=1))
        scr = early.enter_context(tc.tile_pool(name="scr", bufs=5))
        statp = early.enter_context(tc.tile_pool(name="statp", bufs=4))

        # ---- constants into SBUF (two blob DMAs) ----
        cb = consts.tile([128, dram["NBF"]], BF16, tag="cb", name="cb")
        nc.sync.dma_start(out=cb, in_=dram["cbf"][:])
        o = 0
        c_win = cb[:, o:o + H * C].rearrange("p (h c) -> p h c", h=H); o += H * C
        c_wom = cb[:, o:o + H * 2 * G * K].rearrange("p (h c) -> p h c", h=H)
        o += H * 2 * G * K
        c_wout = cb[:, o:o + H * C].rearrange("p (h c) -> p h c", h=H); o += H * C
        c_dconv = cb[:, o:o + K * H * 128].rearrange(
            "p (k h c) -> p k h c", k=K, h=H); o += K * H * 128
        c_ones = cb[:, o:o + 128]; o += 128
        c_one1 = cb[0:1, o:o + 128]; o += 128
        c_E = cb[:, o:o + dram["NE"]].rearrange("p (i c) -> p i c", c=128)
        o += dram["NE"]
        cf = consts.tile([128, dram["NF3"]], F32, tag="cf", name="cf")
        nc.sync.dma_start(out=cf, in_=dram["cf3"][:])
        o = 0
        c_bomd = cf[:, o:o + 2 * G * K]; o += 2 * G * K
        o += T  # (liota slot, unused)
        c_edge = cf[:, o:o + 2]; o += 2
        c_eps = consts.tile([128, 1], F32, tag="c_eps", name="c_eps")
        nc.vector.memset(c_eps, LN_EPS)
        if flags["has_bin"]:
            c_bin = consts.tile([128, H], F32, tag="c_bin", name="c_bin")
            nc.sync.dma_start(out=c_bin, in_=dram["binc"][:])
        if flags["has_convb"]:
            c_convb = consts.tile([128, H, 1], F32, tag="c_convb", name="c_convb")
            nc.sync.dma_start(out=c_convb, in_=dram["convb"][:])
        if flags["has_ln"]:
            c_lngb = consts.tile([128, H, 2], BF16, tag="c_lngb", name="c_lngb")
            nc.sync.dma_start(out=c_lngb, in_=dram["lngb"][:])
        if flags["has_bout"]:
            c_bout = consts.tile([128, H], F32, tag="c_bout", name="c_bout")
            nc.sync.dma_start(out=c_bout, in_=dram["boutc"][:])

        # ---- load x (bf16) and batched-transpose to xT[h] = [128c, L+2] ----
        xvw = dram["xbf"].rearrange("(t p) (h c) -> p h t c", p=128, c=128)
        XO = 128  # 256B-aligned halo offset (xbar transpose needs alignment)
        xT = []
        xbfs = []
        for h in range(H):
            x_bf = b16.tile([128, T, 128], BF16, tag="xbf", name=f"x_bf{h}",
                            bufs=2)
            xbfs.append(x_bf)
            t_ = xTp.tile([128, L + XO + 128], BF16, tag="xT", name=f"xT{h}")
            nc.vector.memset(t_[:, XO - 1:XO], 0.0)
            nc.vector.memset(t_[:, XO + L:XO + L + 1], 0.0)
            xT.append(t_)
        for q in range(2):
            tq = slice(q * (T // 2), (q + 1) * (T // 2))
            for h in range(H):
                eng = nc.sync if h == 0 else nc.scalar
                eng.dma_start(out=xbfs[h][:, tq, :], in_=xvw[:, h, tq, :])
                eng.dma_start_transpose(
                    out=xT[h][:, XO + q * (L // 2):XO + (q + 1) * (L // 2)]
                    .rearrange("c (t p) -> c t p", p=128),
                    in_=xbfs[h][:, tq, :],
                )

        # ---- depthwise conv + LN stats, 1024-col psum chunks ----
        NCH = 4
        CW = 1024
        yb = [arena.tile([128, L], BF16, tag="a8", name=f"y{h}") for h in range(H)]
        featT = [arena.tile([128, L], BF16, tag="a8", name=f"featT{h}")
                 for h in range(H)]
        for n in range(NCH):
            sl = slice(n * CW, (n + 1) * CW)
            ysqc = []
            for h in range(H):
                ps = psp.tile([128, CW], F32, tag="ps", name="ps_y")
                for j in range(K):
                    for q in range(2):
                        nc.tensor.matmul(
                            ps[:, q * 512:(q + 1) * 512],
                            lhsT=c_dconv[:, j, h, :],
                            rhs=xT[h][:, XO + n * CW + q * 512 + j - 1:
                                      XO + n * CW + q * 512 + j + 511],
                            start=(j == 0), stop=(j == K - 1),
                        )
                if flags["has_convb"]:
                    nc.scalar.activation(out=yb[h][:, sl], in_=ps,
                                         func=Act.Identity, bias=c_convb[:, h, :])
                else:
                    nc.scalar.activation(out=yb[h][:, sl], in_=ps, func=Act.Copy)
                yq = ysqp.tile([128, CW], BF16, tag="ysqc", name="ysqc")
                nc.gpsimd.tensor_tensor(out=yq, in0=yb[h][:, sl],
                                        in1=yb[h][:, sl], op=Alu.mult)
                ysqc.append(yq)
            psm = psp.tile([128, CW], F32, tag="ps", name="ps_mu")
            for q in range(2):
                for h in range(H):
                    nc.tensor.matmul(psm[:, q * 512:(q + 1) * 512], lhsT=c_ones,
                                     rhs=yb[h][:, n * CW + q * 512:
                                               n * CW + (q + 1) * 512],
                                     start=(h == 0), stop=(h == H - 1))
            pss = psp.tile([128, CW], F32, tag="ps", name="ps_sq")
            for q in range(2):
                for h in range(H):
                    nc.tensor.matmul(pss[:, q * 512:(q + 1) * 512], lhsT=c_ones,
                                     rhs=ysqc[h][:, q * 512:(q + 1) * 512],
                                     start=(h == 0), stop=(h == H - 1))
            vc = statp.tile([1, CW], F32, tag="sc", name="vc", bufs=2)
            nc.scalar.activation(out=vc, in_=psm[0:1, :], func=Act.Square)
            nc.vector.tensor_tensor(out=vc, in0=pss[0:1, :],
                                    in1=vc, op=Alu.subtract)
            nc.scalar.activation(out=vc, in_=vc, func=Act.Sqrt, bias=c_eps[0:1, :])
            rc = statp.tile([1, CW], F32, tag="sc", name="rc", bufs=2)
            nc.vector.reciprocal_approx_fast(out=rc, in_=vc)
            rbf = statp.tile([1, CW], BF16, tag="scb", name="rbf", bufs=2)
            nc.vector.tensor_copy(out=rbf, in_=rc)
            mbf = statp.tile([1, CW], BF16, tag="scb", name="mbf", bufs=2)
            nc.vector.tensor_mul(mbf, psm[0:1, :], rc)

            # featT = gelu(y*rstd - m2), rstd/m2 broadcast via ones-matmul
            psr = psp.tile([128, CW], F32, tag="ps", name="ps_r")
            for q in range(2):
                nc.tensor.matmul(psr[:, q * 512:(q + 1) * 512], lhsT=c_one1,
                                 rhs=rbf[:, q * 512:(q + 1) * 512],
                                 start=True, stop=True)
            rsb = ysqp.tile([128, CW], BF16, tag="ysqc", name="rsb")
            nc.scalar.activation(out=rsb, in_=psr, func=Act.Copy)
            psr2 = psp.tile([128, CW], F32, tag="ps", name="ps_m2")
            for q in range(2):
                nc.tensor.matmul(psr2[:, q * 512:(q + 1) * 512], lhsT=c_one1,
                                 rhs=mbf[:, q * 512:(q + 1) * 512],
                                 start=True, stop=True)
            m2b = ysqp.tile([128, CW], BF16, tag="ysqc", name="m2b")
            nc.scalar.activation(out=m2b, in_=psr2, func=Act.Copy)
            for h in range(H):
                ztc = ysqp.tile([128, CW], BF16, tag="ysqc", name="ztc")
                nc.vector.tensor_mul(ztc, yb[h][:, sl], rsb)
                nc.vector.tensor_sub(ztc, ztc, m2b)
                if flags["has_ln"]:
                    nc.vector.tensor_scalar(out=ztc, in0=ztc,
                                            scalar1=c_lngb[:, h, 0:1],
                                            scalar2=c_lngb[:, h, 1:2],
                                            op0=Alu.mult, op1=Alu.add)
                nc.scalar.activation(out=featT[h][:, sl], in_=ztc, func=Act.Gelu)

        # ---- omT[q, l] = (feat @ [W_off|W_mask])^T : W_om stationary ----
        GK = G * K
        QOM = 2 * GK
        omT = pers.tile([48, L], BF16, tag="omT", name="omT")
        for ch in range(8):
            pso = psp.tile([128, 512], F32, tag="ps", name="ps_om")
            po = pso[0:QOM, :]
            for h in range(H):
                nc.tensor.matmul(
                    po, lhsT=c_wom[:, h, :],
                    rhs=featT[h][:, ch * 512:(ch + 1) * 512],
                    start=(h == 0), stop=(h == H - 1),
                )
            nc.scalar.activation(out=omT[:, ch * 512:(ch + 1) * 512], in_=po,
                                 func=Act.Copy)
        # transpose back to l-layout: om_l[p, t, q] (bf16)
        om_l = pers.tile([128, T, 48], BF16, tag="om_l", name="om_l")
        nc.sync.dma_start_transpose(
            out=om_l.rearrange("p t c -> p t c"),
            in_=omT.rearrange("q (t p) -> q t p", p=128),
        )

        def rep_t(cst, width):
            return bass.AP(tensor=cst.tensor, offset=cst.offset,
                           ap=[cst.ap[0], [0, T], [1, width]])

        # ---- off = om_l[:,:,0:24] + (b_off + dgrid) ; mask softmax over K ----
        off = scr.tile([128, T, GK], F32, tag="s24", name="off")
        nc.vector.tensor_tensor(out=off, in0=om_l[:, :, 0:GK],
                                in1=rep_t(c_bomd[:, 0:GK], GK), op=Alu.add)
        msk = scr.tile([128, T, GK], F32, tag="s24", name="msk")
        nc.vector.tensor_tensor(out=msk, in0=om_l[:, :, GK:QOM],
                                in1=rep_t(c_bomd[:, GK:QOM], GK), op=Alu.add)
        nc.scalar.activation(out=msk, in_=msk, func=Act.Exp)
        mko = pers.tile([128, T, G], F32, tag="mko", name="mko")
        mkv = msk.rearrange("p t (g k) -> p t g k", k=K)
        nc.vector.tensor_reduce(out=mko, in_=mkv, axis=mybir.AxisListType.X,
                                op=Alu.add)
        mks = pers.tile([128, T, G], F32, tag="mks", name="mks")
        nc.vector.reciprocal_approx_fast(out=mks, in_=mko)
        mskb = scr.tile([128, T, GK], BF16, tag="s24b", name="mskb", bufs=8)
        mbc = bass.AP(tensor=mks.tensor, offset=mks.offset,
                      ap=[mks.ap[0], [G, T], [1, G], [0, K]])
        nc.vector.tensor_tensor(out=mskb.rearrange("p t (g k) -> p t g k", k=K),
                                in0=mkv, in1=mbc, op=Alu.mult)

        # ---- floor(delta), w1 (bf16 weight math; 4 edge fixups) ----
        fi = scr.tile([128, T, GK], I32, tag="s24", name="fi")
        nc.vector.tensor_copy(out=fi, in_=off)
        ff = scr.tile([128, T, GK], F32, tag="s24", name="ff")
        nc.vector.tensor_copy(out=ff, in_=fi)
        fgt = scr.tile([128, T, GK], F32, tag="s24", name="fgt")
        nc.vector.tensor_tensor(out=fgt, in0=ff, in1=off, op=Alu.is_gt)
        nc.vector.tensor_sub(ff, ff, fgt)
        w1 = scr.tile([128, T, GK], BF16, tag="s24b", name="w1", bufs=8)
        nc.vector.tensor_sub(w1, off, ff)
        nc.vector.tensor_tensor(out=w1, in0=w1, in1=mskb, op=Alu.mult)
        b0 = scr.tile([128, T, GK], BF16, tag="s24b", name="b0", bufs=8)
        nc.vector.tensor_tensor(out=b0, in0=mskb, in1=w1, op=Alu.subtract)
        # zero-pad edge mask: w1 tap invalid iff floor(loc) == L-1 pre-wrap,
        # which only happens at (l=0, ff=-1), (l=1, ff=-2), (l=L-2, ff=1),
        # (l=L-1, ff=0).  Per-partition edge values (sentinel 99 elsewhere)
        # keep the two fixup ops full-width (engine base-partition must be 0).
        for i, tt in enumerate((0, T - 1)):
            et = scr.tile([128, GK], BF16, tag="etiny", name="etiny", bufs=2)
            nc.vector.tensor_scalar(out=et, in0=ff[:, tt],
                                    scalar1=c_edge[:, i:i + 1], scalar2=None,
                                    op0=Alu.not_equal)
            nc.vector.tensor_tensor(out=w1[:, tt], in0=w1[:, tt],
                                    in1=et, op=Alu.mult)

        # ---- band weights -> abf[p, t, q=g*8+s] (bf16, padded) ----
        abf = pers.tile([128, T, 128], BF16, tag="abf", name="abf")
        nc.vector.memset(
            bass.AP(tensor=abf.tensor, offset=abf.offset + G * NSP,
                    ap=[abf.ap[0], [128, T], [1, 128 - G * NSP]]), 0.0)
        nc.vector.memset(
            bass.AP(tensor=abf.tensor, offset=abf.offset + NS,
                    ap=[abf.ap[0], [128, T], [NSP, G], [1, NSP - NS]]), 0.0)
        eq = {}
        for s in range(SMIN, SMAX):
            e = scr.tile([128, T, GK], BF16, tag="s24b", name=f"eqs{s}", bufs=8)
            nc.vector.tensor_scalar(out=e, in0=ff, scalar1=float(s), scalar2=None,
                                    op0=Alu.is_equal)
            eq[s] = e
        for s in range(SMIN, SMAX + 1):
            cc = scr.tile([128, T, GK], BF16, tag="s24b", name="cc", bufs=8)
            have0 = s in eq
            have1 = (s - 1) in eq
            if have0 and have1:
                c2 = scr.tile([128, T, GK], BF16, tag="s24b", name="c2", bufs=8)
                nc.vector.tensor_tensor(out=cc, in0=b0, in1=eq[s], op=Alu.mult)
                nc.vector.tensor_tensor(out=c2, in0=w1, in1=eq[s - 1], op=Alu.mult)
                nc.vector.tensor_add(cc, cc, c2)
            elif have0:
                nc.vector.tensor_tensor(out=cc, in0=b0, in1=eq[s], op=Alu.mult)
            else:
                nc.vector.tensor_tensor(out=cc, in0=w1, in1=eq[s - 1], op=Alu.mult)
            dst = bass.AP(tensor=abf.tensor, offset=abf.offset + (s - SMIN),
                          ap=[abf.ap[0], [128, T], [NSP, G]])
            nc.vector.tensor_reduce(
                out=dst,
                in_=cc.rearrange("p t (g k) -> p t g k", k=K),
                axis=mybir.AxisListType.X, op=Alu.add,
            )

        early.close()
        wsp = ctx.enter_context(tc.tile_pool(name="wsp", bufs=2))
        ostp = ctx.enter_context(tc.tile_pool(name="ostp", bufs=2))

        # ---- xpT[hp] = (x @ W_in)^T with 2-col wrap halo (c-layout) ----
        # Emitted after the band math in program order so these matmuls fill
        # the tensor-engine idle while the vector engine chews the band ops.
        XHO = 2
        xpT = []
        xpT1 = []
        for hp in range(H):
            t_ = xTp.tile([128, XHO + L + 2], BF16, tag="xpT", name=f"xpT{hp}")
            for ch in range(NCH):
                psx = psp.tile([128, CW], F32, tag="ps", name="ps_xp")
                for q in range(2):
                    for h in range(H):
                        nc.tensor.matmul(
                            psx[:, q * 512:(q + 1) * 512],
                            lhsT=c_win[:, h, hp * 128:(hp + 1) * 128],
                            rhs=xT[h][:, XO + ch * CW + q * 512:
                                      XO + ch * CW + (q + 1) * 512],
                            start=(h == 0), stop=(h == H - 1),
                        )
                if flags["has_bin"]:
                    nc.scalar.activation(out=t_[:, XHO + ch * CW: XHO + (ch + 1) * CW],
                                         in_=psx, func=Act.Identity,
                                         bias=c_bin[:, hp:hp + 1])
                else:
                    nc.scalar.activation(out=t_[:, XHO + ch * CW: XHO + (ch + 1) * CW],
                                         in_=psx, func=Act.Copy)
            # wrap halo: cols [0:2] <- l = L-2..L-1 ; cols [XHO+L:] <- l = 0..1
            nc.vector.tensor_copy(out=t_[:, 0:XHO], in_=t_[:, L:L + XHO])
            nc.vector.tensor_copy(out=t_[:, XHO + L:XHO + L + 2],
                                  in_=t_[:, XHO:XHO + 2])
            xpT.append(t_)
            # odd-shift 4B-aligned copy: xpT1[:, i] = xpT[:, i+1]
            t1 = xTp.tile([128, XHO + L + 2], BF16, tag="xpT1", name=f"xpT1_{hp}")
            nc.scalar.dma_start(out=t1[:, 0:XHO + L + 1], in_=t_[:, 1:XHO + L + 2])
            xpT1.append(t1)

        # ---- abf -> aT[(g,s), l] (128-pad; contraction input for E mms) ----
        aT = pers.tile([128, L], BF16, tag="aT", name="aT")
        for q in range(2):
            tq = slice(q * (T // 2), (q + 1) * (T // 2))
            nc.sync.dma_start_transpose(
                out=aT[:, q * (L // 2):(q + 1) * (L // 2)]
                .rearrange("c (t p) -> c t p", p=128),
                in_=abf[:, tq, :],
            )

        # ---- apply: ws_s = E_hs^T @ aT (one-hot expansion on PE), then
        #      sampT[hp] = sum_s ws_s * xpT[hp] shifted; adds on gpsimd ----
        sampT = [arena.tile([128, L], BF16, tag="a8", name=f"sampT{hp}")
                 for hp in range(H)]
        for hp in range(H):
            ws = []
            for s in range(NS):
                # one LDWEIGHTS, 8 back-to-back matmuls over full L
                w_ = wsp.tile([128, L], BF16, tag="ws", name=f"ws{s}", bufs=5)
                for half in range(L // CW):
                    pse = psp.tile([128, CW], F32, tag="ps", name="pse")
                    for q in range(2):
                        nc.tensor.matmul(
                            pse[:, q * 512:(q + 1) * 512],
                            lhsT=c_E[:, hp * NS + s, :],
                            rhs=aT[:, half * CW + q * 512:
                                   half * CW + (q + 1) * 512],
                            start=True, stop=True,
                        )
                    wdst = w_[:, half * CW:(half + 1) * CW]
                    if s % 2 == 0:
                        nc.scalar.activation(out=wdst, in_=pse, func=Act.Copy)
                    else:
                        nc.vector.tensor_copy(out=wdst, in_=pse)
                ws.append(w_)
            acc = sampT[hp]
            X = xpT[hp]
            X1 = xpT1[hp]
            # shift s-SMIN: xpT col offset = base + s (XHO cancels SMIN)
            nc.vector.tensor_tensor(
                out=acc, in0=ws[0], in1=X[:, 0:L], op=Alu.mult)
            for s in range(1, NS):
                if s % 2 == 1:
                    xop = X1[:, s - 1:s - 1 + L]
                else:
                    xop = X[:, s:s + L]
                tmp = wsp.tile([128, L], BF16, tag="atmp", name="atmp",
                               bufs=2)
                nc.vector.tensor_tensor(out=tmp, in0=ws[s], in1=xop,
                                        op=Alu.mult)
                nc.vector.tensor_tensor(out=acc, in0=acc, in1=tmp, op=Alu.add)

        # ---- out-proj: psP[co, l] = sum_ci Wout sampT ----
        for co in range(H):
            for lk in range(L // CW):
                lo = lk * CW
                psq = psp.tile([128, CW], F32, tag="ps", name="ps_out")
                for q in range(2):
                    for ci in range(H):
                        nc.tensor.matmul(
                            psq[:, q * 512:(q + 1) * 512],
                            lhsT=c_wout[:, ci, co * 128:(co + 1) * 128],
                            rhs=sampT[ci][:, lo + q * 512:lo + (q + 1) * 512],
                            start=(ci == 0), stop=(ci == H - 1),
                        )
                ost = ostp.tile([128, CW], BF16, tag="ost", name="ost")
                if flags["has_bout"]:
                    nc.scalar.activation(out=ost, in_=psq, func=Act.Identity,
                                         bias=c_bout[:, co:co + 1])
                else:
                    nc.scalar.activation(out=ost, in_=psq, func=Act.Copy)
                ov = dram["out_d"].rearrange("(h p) l -> p h l", p=128)
                nc.gpsimd.dma_start(out=ov[:, co, lo:lo + CW], in_=ost)


def _prep_consts(inputs):
    f32 = np.float32
    W_in = np.asarray(inputs["W_in"], f32)
    W_off = np.asarray(inputs["W_off"], f32)
    W_mask = np.asarray(inputs["W_mask"], f32)
    W_out = np.asarray(inputs["W_out"], f32)
    conv_w = np.asarray(inputs["conv_w"], f32)[:, 0, :]      # [C, K]
    b_in = np.asarray(inputs["b_in"], f32)
    conv_b = np.asarray(inputs["conv_b"], f32)
    ln_g = np.asarray(inputs["ln_g"], f32)
    ln_b = np.asarray(inputs["ln_b"], f32)
    b_off = np.asarray(inputs["b_off"], f32)
    b_mask = np.asarray(inputs["b_mask"], f32)
    b_out = np.asarray(inputs["b_out"], f32)

    flags = {
        "has_bin": bool(np.any(b_in != 0)),
        "has_convb": bool(np.any(conv_b != 0)),
        "has_ln": bool(np.any(ln_g != 1) or np.any(ln_b != 0)),
        "has_bout": bool(np.any(b_out != 0)),
    }

    import ml_dtypes
    bf16 = ml_dtypes.bfloat16

    def to_bf(a):
        return a.astype(bf16)

    cm = {}
    bf_parts = []
    bf_parts.append(np.transpose(W_in.reshape(H, 128, C), (1, 0, 2)).reshape(128, -1))
    bf_parts.append(np.transpose(
        np.concatenate([W_off, W_mask], axis=1).reshape(H, 128, 2 * G * K),
        (1, 0, 2)).reshape(128, -1))
    bf_parts.append(np.transpose(W_out.reshape(H, 128, C), (1, 0, 2)).reshape(128, -1))
    dmats = np.zeros((K, H, 128, 128), f32)
    for j in range(K):
        for h in range(H):
            np.fill_diagonal(dmats[j, h], conv_w[h * 128:(h + 1) * 128, j])
    bf_parts.append(np.transpose(dmats, (2, 0, 1, 3)).reshape(128, -1))
    bf_parts.append(np.full((128, 128), 1.0 / C, f32))
    onerow = np.zeros((128, 128), f32)
    onerow[0, :] = 1.0
    bf_parts.append(onerow)
    Em = np.zeros((128, H * NS, 128), f32)
    for hp in range(H):
        for si in range(NS):
            for c in range(128):
                g = hp * (G // H) + c // GC
                Em[g * NSP + si, hp * NS + si, c] = 1.0
    bf_parts.append(Em.reshape(128, -1))
    cm["cbf"] = to_bf(np.concatenate(bf_parts, axis=1))
    f3_parts = []
    dg = np.tile(np.array([-1.0, 0.0, 1.0], f32), G)
    bomv = np.concatenate([b_off + dg, b_mask])
    f3_parts.append(np.tile(bomv[None, :], (128, 1)))
    p = np.arange(128, dtype=f32)[:, None]
    tt = np.arange(T, dtype=f32)[None, :]
    f3_parts.append(tt * 128 + p)
    edge = np.full((128, 2), 99.0, f32)
    edge[0, 0] = -1.0
    edge[1, 0] = -2.0
    edge[126, 1] = 1.0
    edge[127, 1] = 0.0
    f3_parts.append(edge)
    cm["cf3"] = np.concatenate(f3_parts, axis=1).astype(f32)
    if flags["has_bin"]:
        cm["binc"] = np.transpose(b_in.reshape(H, 128), (1, 0)).astype(f32)
    if flags["has_convb"]:
        cm["convb"] = np.transpose(conv_b.reshape(H, 128, 1), (1, 0, 2)).astype(f32)
    if flags["has_ln"]:
        cm["lngb"] = to_bf(np.transpose(
            np.stack([ln_g.reshape(H, 128), ln_b.reshape(H, 128)], axis=-1),
            (1, 0, 2)))
    if flags["has_bout"]:
        cm["boutc"] = np.transpose(b_out.reshape(H, 128), (1, 0)).astype(f32)
    return flags, cm, bf16


def kernel(**inputs):
    x = np.asarray(inputs["x"], np.float32)
    flags, cm, bf16 = _prep_consts(inputs)

    key = tuple(sorted(flags.items()))
    if key not in _CACHE:
        _CACHE[key] = _build(flags)
    nc = _CACHE[key]

    in_maps = []
    for n in range(N):
        m = dict(cm)
        m["xbf"] = x[n].astype(bf16)
        in_maps.append(m)
    res = run_bass_kernel_spmd(nc, in_maps, core_ids=list(range(N)))
    out = np.stack([np.asarray(res.results[n]["out"], np.float32).T
                    for n in range(N)], axis=0)
    return out
